# revision 13
# baseline (speedup 1.0000x reference)
"""DropToken gather kernel for Trainium2 (8 NeuronCores).

Computes out[b, c, :] = inputs[b, idx[c], :] (the reference's one-hot
matmul is just a row gather). Memory-bound: per core 8 MB gathered read
+ 8 MB contiguous write.

Sharding: core k -> batch b = k//2, cap-half h = k%2. Each core gathers
2048 rows of 4 KB from its batch's [8192, 1024] slice. Indices are
reshaped host-side to [128, T] so row r = p*T + t lands in partition p,
free-dim slot t; the store to DRAM is then fully contiguous.
"""

import numpy as np

import concourse.bass as bass
import concourse.tile as tile
from concourse import bacc, mybir
from concourse.bass_utils import run_bass_kernel_spmd

B = 4
LENGTH = 8192
EMBED = 1024
CAP = 4096
N_CORES = 8
ROWS_PER_CORE = B * CAP // N_CORES  # 2048
T = ROWS_PER_CORE // 128  # 16 gathered rows per partition

_nc_cache = None
USE_TILE = True
STRIP_INIT_BARRIER = True


def _strip_init_barrier(nc):
    """Remove the Bass-init const memsets and all-engine barrier from the
    entry block. This kernel has no cross-engine deps besides DMA
    semaphores (runtime-zeroed at NEFF load), so engine-boot alignment is
    unnecessary; saves ~3us of startup."""
    import concourse.mybir as mybir

    blk = nc.m.functions[0].blocks[0]
    blk.instructions = [
        ins
        for ins in blk.instructions
        if not isinstance(
            ins, (mybir.InstMemset, mybir.InstDrain, mybir.InstEventSemaphore)
        )
    ]


def _build_nc_tile():
    nc = bacc.Bacc("TRN2", target_bir_lowering=False, debug=False, num_devices=N_CORES)
    x = nc.dram_tensor("x", [LENGTH, EMBED], mybir.dt.float32, kind="ExternalInput").ap()
    idx = nc.dram_tensor("idx", [128, T], mybir.dt.int32, kind="ExternalInput").ap()
    out = nc.dram_tensor(
        "out", [128, T * EMBED], mybir.dt.float32, kind="ExternalOutput"
    ).ap()

    with tile.TileContext(nc) as tc:
        with (
            tc.tile_pool(name="idxp", bufs=1) as idxp,
            tc.tile_pool(name="io", bufs=16) as io,
        ):
            idx_tile = idxp.tile([128, T], mybir.dt.int32)
            nc.sync.dma_start(out=idx_tile[:], in_=idx[:, :])
            for t in range(T):
                g = io.tile([128, EMBED], mybir.dt.float32, tag="g")
                nc.gpsimd.indirect_dma_start(
                    out=g[:],
                    out_offset=None,
                    in_=x[:, :],
                    in_offset=bass.IndirectOffsetOnAxis(
                        ap=idx_tile[:, t : t + 1], axis=0
                    ),
                )
                nc.sync.dma_start(
                    out=out[:, t * EMBED : (t + 1) * EMBED], in_=g[:]
                )
    if STRIP_INIT_BARRIER:
        _strip_init_barrier(nc)
    nc.compile()
    return nc


def _build_nc_raw():
    """Raw bacc with manual semaphores: no Tile scheduling preamble/tail.

    gpsimd: 16 indirect gathers back-to-back (dedicated SBUF slot each, no
    WAR waits), cumulative completion sem. sync: idx load up front, then
    store t as soon as gather t's transfer lands; final wait for all
    stores. Cumulative sem thresholds are safe: every DMA on a queue
    spreads over all 16 SDMA engines which each drain FIFO, so the sem
    reaching 16*(t+1) implies gathers 0..t fully landed.
    """
    nc = bacc.Bacc("TRN2", target_bir_lowering=False, debug=False, num_devices=N_CORES)
    x = nc.dram_tensor("x", [LENGTH, EMBED], mybir.dt.float32, kind="ExternalInput").ap()
    idx = nc.dram_tensor("idx", [128, T], mybir.dt.int32, kind="ExternalInput").ap()
    out = nc.dram_tensor(
        "out", [128, T * EMBED], mybir.dt.float32, kind="ExternalOutput"
    ).ap()

    from contextlib import ExitStack

    NSEM = 8
    with ExitStack() as ctx:
        idx_tile = ctx.enter_context(nc.sbuf_tensor([128, T], mybir.dt.int32))
        gbuf = ctx.enter_context(
            nc.sbuf_tensor([128, T * EMBED], mybir.dt.float32)
        )
        isem = ctx.enter_context(nc.semaphore("isem"))
        ssem = ctx.enter_context(nc.semaphore("ssem"))
        gsems = [ctx.enter_context(nc.semaphore(f"gsem{i}")) for i in range(NSEM)]
        block = ctx.enter_context(nc.Block())

        @block.sync
        def _(sync):
            sync.dma_start(out=idx_tile[:, :], in_=idx[:, :]).then_inc(isem, 16)
            for t in range(T):
                sync.wait_ge(gsems[t % NSEM], 16 * (t // NSEM + 1))
                sync.dma_start(
                    out=out[:, t * EMBED : (t + 1) * EMBED],
                    in_=gbuf[:, t * EMBED : (t + 1) * EMBED],
                ).then_inc(ssem, 16)
            sync.wait_ge(ssem, 16 * T)

        @block.gpsimd
        def _(gpsimd):
            gpsimd.wait_ge(isem, 16)
            for t in range(T):
                gpsimd.indirect_dma_start(
                    out=gbuf[:, t * EMBED : (t + 1) * EMBED],
                    out_offset=None,
                    in_=x[:, :],
                    in_offset=bass.IndirectOffsetOnAxis(
                        ap=idx_tile[:, t : t + 1], axis=0
                    ),
                ).then_inc(gsems[t % NSEM], 16)

    nc.compile()
    return nc


def _build_nc():
    global _nc_cache
    if _nc_cache is None:
        _nc_cache = _build_nc_tile() if USE_TILE else _build_nc_raw()
    return _nc_cache


def _shard_inputs(inputs: np.ndarray, idx: np.ndarray):
    in_maps = []
    half = CAP // 2
    for k in range(N_CORES):
        b, h = divmod(k, 2)
        shard = np.ascontiguousarray(
            idx[h * half : (h + 1) * half].reshape(128, T).astype(np.int32)
        )
        in_maps.append({"x": np.ascontiguousarray(inputs[b]), "idx": shard})
    return in_maps


def _run(inputs: np.ndarray, idx: np.ndarray, **run_kwargs):
    nc = _build_nc()
    in_maps = _shard_inputs(inputs, idx)
    res = run_bass_kernel_spmd(nc, in_maps, list(range(N_CORES)), **run_kwargs)
    half = CAP // 2
    out = np.empty((B, CAP, EMBED), np.float32)
    for k in range(N_CORES):
        b, h = divmod(k, 2)
        out[b, h * half : (h + 1) * half] = res.results[k]["out"].reshape(
            ROWS_PER_CORE, EMBED
        )
    return out, res


def kernel(inputs: np.ndarray, idx: np.ndarray) -> np.ndarray:
    inputs = np.asarray(inputs, dtype=np.float32)
    idx = np.asarray(idx, dtype=np.int32)
    out, _ = _run(inputs, idx)
    return out
